# revision 4
# baseline (speedup 1.0000x reference)
"""MultiHeadGAT Trainium2 kernel (8 NeuronCores, data-parallel over batch).

Reference computation (per batch b of 32, n=512 nodes, d=128 feats, H=8 heads,
HID=64, top-k=16, leaky=0.2):
    h' = (h @ W).reshape(n, H, HID)                      # projection
    ei[g,i] = h'[i,g,:] . a_i[g];  ej[g,j] = h'[j,g,:] . a_j[g]
    e[g,i,j] = leaky_relu(ei[g,i] + ej[g,j])
    mask = topk_16(e, axis=j) | eye(n)
    attn = softmax(where(mask, e, -1e9))
    out = elu(attn @ h')

Key structural facts exploited:
  * leaky_relu is strictly monotone, and e[g,i,:] = leaky(ei[g,i] + ej[g,:]),
    so the top-16 column set J_g is THE SAME for every row i: it is the
    top-16 of the ej[g,:] vector. The attention matrix is therefore
    rank-17-structured: 16 shared columns + the diagonal.
  * softmax rows reduce to 17 candidates; -1e9 fills underflow to exact 0
    in f32, so computing only the 17 candidates is exact.
  * exp(leaky(s)) = max(exp(s), exp(0.2*s))  (monotonicity), so the leaky
    never needs its own pass: two scalar-engine Exp ACTs + one vector max.
  * elu(y) = max(y, exp(min(y,0)) - 1)  (exact identity).

This version is a heavily fused rewrite of the first working kernel:
stage-C elementwise work is batched into single instructions across all
4 batches (cuts instruction count and semaphore traffic ~4x), pex/qn run
in bf16 (2x DVE modes), the elementwise load is spread across Vector /
Scalar / GpSimd, and the epilogue is fused per half-batch.
"""
import sys
import numpy as np

sys.path.insert(0, "/opt/trn_rl_repo")

from contextlib import ExitStack

import concourse.bass as bass
import concourse.tile as tile
from concourse import bacc, mybir
from concourse.bass_utils import run_bass_kernel_spmd

f32 = mybir.dt.float32
bf16 = mybir.dt.bfloat16
AX = mybir.AxisListType
ALU = mybir.AluOpType
AF = mybir.ActivationFunctionType

N_HEADS = 8
HID = 64
TOP_K = 16
SLOPE = 0.2
BS, N, D = 32, 512, 128
CORES = 8
BPC = BS // CORES          # batches per core = 4
NCH = N // 128             # n-chunks = 4
GD = N_HEADS * HID         # 512


def _mid_bcast(ap, insert_at, counts_steps):
    """Insert [step, count] dims into an AP at position insert_at."""
    new = list(ap.ap)
    for step, count in reversed(counts_steps):
        new.insert(insert_at, [step, count])
    return bass.AP(ap.tensor, ap.offset, new)


def build_graph():
    nc = bacc.Bacc("TRN2", target_bir_lowering=False, debug=False)

    hT_ext = nc.dram_tensor("hT", [BPC, D, N], f32, kind="ExternalInput")
    hTb_ext = nc.dram_tensor("hTb", [BPC, D, N], bf16, kind="ExternalInput")
    W_ext = nc.dram_tensor("W", [D, GD], bf16, kind="ExternalInput")
    P_ext = nc.dram_tensor("P", [D, 16], f32, kind="ExternalInput")
    out_ext = nc.dram_tensor("out", [BPC, N, N_HEADS, HID], bf16,
                             kind="ExternalOutput")
    hT = hT_ext.ap()
    hTb = hTb_ext.ap()
    Wap = W_ext.ap()
    Pap = P_ext.ap()
    outap = out_ext.ap()

    with tile.TileContext(nc) as tc, ExitStack() as ctx:
        const = ctx.enter_context(tc.tile_pool(name="const", bufs=1))
        sb = ctx.enter_context(tc.tile_pool(name="sb", bufs=2))
        ps = ctx.enter_context(tc.tile_pool(name="ps", bufs=2, space="PSUM"))

        # ---------------- constants ----------------
        W_sb = const.tile([128, GD], bf16)
        nc.sync.dma_start(W_sb[:], Wap)
        P_sb = const.tile([128, 16], f32)
        nc.sync.dma_start(P_sb[:], Pap)

        rowi = const.tile([128, 128], f32)
        nc.gpsimd.iota(rowi[:], [[1, 128]], channel_multiplier=0,
                       allow_small_or_imprecise_dtypes=True)
        coli = const.tile([128, 1], f32)
        nc.gpsimd.iota(coli[:], [[0, 1]], channel_multiplier=1,
                       allow_small_or_imprecise_dtypes=True)
        ident = const.tile([128, 128], f32)
        nc.vector.tensor_scalar(ident[:], rowi[:], coli[:], None,
                                op0=ALU.is_equal)

        identb = const.tile([128, 128], bf16)
        nc.vector.tensor_copy(identb[:], ident[:])

        ones32 = const.tile([32, 128], f32)
        nc.gpsimd.memset(ones32[:], 1.0)

        # block-diag mask: mblk[p, f] = (16*(f//64) <= p <= 16*(f//64)+15)
        colg_lo = const.tile([128, GD], f32)
        nc.gpsimd.iota(colg_lo[:].rearrange("p (g d) -> p g d", g=N_HEADS),
                       [[16, N_HEADS], [0, HID]], channel_multiplier=0,
                       allow_small_or_imprecise_dtypes=True)
        colg_hi = const.tile([128, GD], f32)
        nc.gpsimd.iota(colg_hi[:].rearrange("p (g d) -> p g d", g=N_HEADS),
                       [[16, N_HEADS], [0, HID]], base=15, channel_multiplier=0,
                       allow_small_or_imprecise_dtypes=True)
        mlo = const.tile([128, GD], f32)
        nc.vector.tensor_scalar(mlo[:], colg_lo[:], coli[:], None,
                                op0=ALU.is_le)
        mhi = const.tile([128, GD], f32)
        nc.vector.tensor_scalar(mhi[:], colg_hi[:], coli[:], None,
                                op0=ALU.is_ge)
        mblk = const.tile([128, GD], f32)
        nc.vector.tensor_tensor(mblk[:], mlo[:], mhi[:], op=ALU.mult)

        T = const.tile([32, N], f32)          # ej rows: (b,g) x n
        T2 = const.tile([32, N], f32)
        vals = const.tile([32, 16], f32)

        # persistent (whole-kernel) SBUF state
        hp_all = const.tile([128, BPC, NCH, GD], bf16)    # h' per batch
        eij_all = const.tile([128, BPC, NCH, 16], f32)    # [ej(0:8)|ei(8:16)]

        # ---------------- stage A: projection + scores ----------------
        for b in range(BPC):
            ht = sb.tile([128, N], f32, tag="ht")
            nc.sync.dma_start(ht[:], hT[b])
            htb = sb.tile([128, N], bf16, tag="htb")
            nc.sync.dma_start(htb[:], hTb[b])

            for p in range(2):                 # chunk pairs (0,1) and (2,3)
                hp_ps = ps.tile([128, 2, GD], f32, tag="big")
                for i in range(2):
                    c = 2 * p + i
                    nc.tensor.matmul(hp_ps[:, i, :],
                                     htb[:, c * 128:(c + 1) * 128],
                                     W_sb[:], start=True, stop=True)
                if b % 2 == 0:
                    nc.scalar.copy(hp_all[:, b, 2 * p:2 * p + 2, :], hp_ps[:])
                else:
                    nc.vector.tensor_copy(hp_all[:, b, 2 * p:2 * p + 2, :],
                                          hp_ps[:])

            # scores: EIJT[(ij,g), n] fp32; per-node layout via PE transpose
            # of the SAME values (bit-identity matters for the one-hot gather)
            eijt_ps = ps.tile([16, N], f32, tag="sm")
            nc.tensor.matmul(eijt_ps[:], P_sb[:], ht[:], start=True, stop=True)
            ejt16 = sb.tile([16, N], f32, tag="ejt16")
            nc.scalar.copy(ejt16[:], eijt_ps[:])
            nc.sync.dma_start(T[b * 8:(b + 1) * 8, :], ejt16[0:8, :])

            eij_ps = ps.tile([128, NCH, 16], f32, tag="sm")
            for c in range(NCH):
                nc.tensor.transpose(eij_ps[:, c, :],
                                    ejt16[:, c * 128:(c + 1) * 128],
                                    ident[0:16, 0:16])
            nc.vector.tensor_copy(eij_all[:, b], eij_ps[:])

        # ---------------- stage B: top-16 of ej per (b,g) ----------------
        nc.vector.max(vals[:, 0:8], T[:])
        nc.vector.match_replace(T2[:], vals[:, 0:8], T[:], -1e30)
        nc.vector.max(vals[:, 8:16], T2[:])

        # ---------------- stage C: attention (fused across batches) -----
        # broadcast vals to all 128 partitions: vbc[p, (b,g,c)] = vals[8b+g, c]
        rhsb = const.tile([32, BPC, N_HEADS, 16], f32)
        vals_mid = _mid_bcast(vals[:, 0:16], 1, [[0, BPC], [0, N_HEADS]])
        id_bg = ident[0:32, 0:32].rearrange(
            "p (b g) -> p b g", b=BPC).broadcast_to([32, BPC, N_HEADS, 16])
        nc.vector.tensor_tensor(rhsb[:], vals_mid, id_bg, op=ALU.mult)
        vbc_ps = ps.tile([128, BPC * 128], f32, tag="sm")
        nc.tensor.matmul(vbc_ps[:], ones32[:],
                         rhsb[:].rearrange("k b g c -> k (b g c)"),
                         start=True, stop=True)
        vbc = const.tile([128, BPC, N_HEADS, 16], f32)
        nc.vector.tensor_copy(vbc[:], vbc_ps[:])

        # candidate scores s = ej_topk + ei:  [128, b, c, g, 16]
        cand16 = const.tile([128, BPC, NCH, N_HEADS, 16], f32)
        vbc_rep = _mid_bcast(vbc[:], 2, [[0, NCH]])
        ei_bc = eij_all[:, :, :, 8:16].broadcast_to(
            [128, BPC, NCH, N_HEADS, 16])
        nc.gpsimd.tensor_tensor(cand16[:], vbc_rep, ei_bc, op=ALU.add)
        candd = const.tile([128, BPC, NCH, N_HEADS], f32)
        nc.vector.tensor_tensor(candd[:], eij_all[:, :, :, 0:8],
                                eij_all[:, :, :, 8:16], op=ALU.add)

        # pex = exp(leaky(s)) = max(exp(s), exp(0.2 s))   (bf16)
        pexA = const.tile([128, BPC, NCH, N_HEADS, 16], bf16)
        nc.scalar.activation(pexA[:], cand16[:], AF.Exp)
        pexB = const.tile([128, BPC, NCH, N_HEADS, 16], bf16)
        nc.scalar.activation(pexB[:], cand16[:], AF.Exp, scale=SLOPE)
        pex = const.tile([128, BPC, NCH, N_HEADS, 16], bf16)
        nc.vector.tensor_tensor(pex[:], pexA[:], pexB[:], op=ALU.max)

        pexdA = const.tile([128, BPC, NCH, N_HEADS], bf16)
        nc.scalar.activation(pexdA[:], candd[:], AF.Exp)
        pexdB = const.tile([128, BPC, NCH, N_HEADS], bf16)
        nc.scalar.activation(pexdB[:], candd[:], AF.Exp, scale=SLOPE)
        pexd = const.tile([128, BPC, NCH, N_HEADS], bf16)
        nc.vector.tensor_tensor(pexd[:], pexdA[:], pexdB[:], op=ALU.max)

        # diagonal indicator: i not in J_g  <=>  ej_i < t_g (16th largest)
        ind = const.tile([128, BPC, NCH, N_HEADS], f32)
        t_bc = _mid_bcast(
            bass.AP(vbc[:].tensor, vbc[:].offset + 15,
                    [vbc[:].ap[0], [128, BPC], [16, N_HEADS]]), 2, [[0, NCH]])
        nc.vector.tensor_tensor(ind[:], eij_all[:, :, :, 0:8], t_bc,
                                op=ALU.is_lt)
        pdiag = const.tile([128, BPC, NCH, N_HEADS], f32)
        nc.vector.tensor_tensor(pdiag[:], pexd[:], ind[:], op=ALU.mult)

        # softmax denominator over the 17 candidates
        den = const.tile([128, BPC, NCH, N_HEADS], f32)
        nc.vector.tensor_reduce(den[:], pex[:], axis=AX.X, op=ALU.add)
        den2 = const.tile([128, BPC, NCH, N_HEADS], f32)
        nc.vector.tensor_tensor(den2[:], den[:], pdiag[:], op=ALU.add)
        recip = const.tile([128, BPC, NCH, N_HEADS], f32)
        nc.vector.reciprocal(recip[:], den2[:])
        recipb = const.tile([128, BPC, NCH, N_HEADS], bf16)
        nc.vector.tensor_copy(recipb[:], recip[:])

        qn = const.tile([128, BPC, NCH, N_HEADS, 16], bf16)
        nc.vector.tensor_tensor(
            qn[:], pex[:],
            recipb[:].broadcast_to([128, BPC, NCH, N_HEADS, 16]), op=ALU.mult)
        pdn = const.tile([128, BPC, NCH, N_HEADS], bf16)
        nc.vector.tensor_tensor(pdn[:], pdiag[:], recipb[:], op=ALU.mult)

        # one-hot S[n, (b,c,g,c16)] = (ej[n,g] == vals[8b+g, c16])
        S = const.tile([128, BPC, NCH, N_HEADS, 16], bf16)
        ej_bc = eij_all[:, :, :, 0:8].broadcast_to(
            [128, BPC, NCH, N_HEADS, 16])
        nc.vector.tensor_tensor(S[:], ej_bc, vbc_rep, op=ALU.is_equal)

        # ---------------- per-batch attention apply ----------------
        for b in range(BPC):
            # gathered rows H_gath[(g,c), :] = h'[j_gc, :], then block-mask
            hg_ps = ps.tile([128, GD], f32, tag="hg")
            for c in range(NCH):
                nc.tensor.matmul(
                    hg_ps[:],
                    S[:, b, c].rearrange("p g c -> p (g c)"),
                    hp_all[:, b, c, :],
                    start=(c == 0), stop=(c == NCH - 1))
            hblk = sb.tile([128, GD], bf16, tag="hblk")
            nc.vector.tensor_tensor(hblk[:], hg_ps[:], mblk[:], op=ALU.mult)

            # qT via PE transpose
            qt_ps = ps.tile([128, NCH, 128], bf16, tag="sm")
            for c in range(NCH):
                nc.tensor.transpose(
                    qt_ps[:, c, :],
                    qn[:, b, c].rearrange("p g c -> p (g c)"), identb[:])
            qt = sb.tile([128, NCH, 128], bf16, tag="qt")
            nc.scalar.copy(qt[:], qt_ps[:])

            # diagonal term dt = h' * pdn (row scale)
            dt = sb.tile([128, NCH, N_HEADS, HID], bf16, tag="dt")
            hp_b = hp_all[:, b].rearrange("p c (g d) -> p c g d", g=N_HEADS)
            pdn_bc = pdn[:, b].broadcast_to([128, NCH, N_HEADS, HID])
            if b < 2:
                nc.vector.tensor_tensor(dt[:], hp_b, pdn_bc, op=ALU.mult)
            else:
                nc.gpsimd.tensor_tensor(dt[:], hp_b, pdn_bc, op=ALU.mult)

            # attention output + diagonal, then ELU (per half-batch)
            ot = sb.tile([128, NCH, GD], bf16, tag="ot")
            for h in range(2):
                o_ps = ps.tile([128, 2, GD], f32, tag="big")
                for i in range(2):
                    c = 2 * h + i
                    nc.tensor.matmul(o_ps[:, i, :], qt[:, c, :], hblk[:],
                                     start=True, stop=False)
                    nc.tensor.matmul(
                        o_ps[:, i, :], identb[:],
                        dt[:, c].rearrange("p g d -> p (g d)"),
                        start=False, stop=True)
                # elu(y) = max(y, exp(min(y,0)) - 1)
                m = sb.tile([128, 2, GD], bf16, tag="m")
                if b % 2 == 0:
                    nc.vector.tensor_scalar_min(m[:], o_ps[:], 0.0)
                    vex = sb.tile([128, 2, GD], bf16, tag="vex")
                    nc.scalar.activation(vex[:], m[:], AF.Exp)
                else:
                    nc.scalar.activation(m[:], o_ps[:], AF.Relu, scale=-1.0)
                    vex = sb.tile([128, 2, GD], bf16, tag="vex")
                    nc.scalar.activation(vex[:], m[:], AF.Exp, scale=-1.0)
                nc.vector.scalar_tensor_tensor(
                    ot[:, 2 * h:2 * h + 2, :], vex[:], 1.0, o_ps[:],
                    op0=ALU.subtract, op1=ALU.max)
            nc.sync.dma_start(
                outap[b].rearrange("(c p) g d -> p c g d", c=NCH),
                ot[:].rearrange("p c (g d) -> p c g d", g=N_HEADS))

    nc.compile()
    return nc


_CACHE = {}


def _get_graph():
    if "nc" not in _CACHE:
        _CACHE["nc"] = build_graph()
    return _CACHE["nc"]


def _prep_inputs(h, W, att_a):
    """Host-side marshalling: shard h over cores, transpose to [b,d,n],
    fold attention vectors into P = [W_g @ a_j_g | W_g @ a_i_g]."""
    h = np.asarray(h, dtype=np.float32)
    W = np.asarray(W, dtype=np.float32)
    att_a = np.asarray(att_a, dtype=np.float32)
    P = np.empty((D, 16), dtype=np.float32)
    for g in range(N_HEADS):
        Wg = W[:, g * HID:(g + 1) * HID]
        P[:, g] = Wg @ att_a[g, HID:]      # a_j -> ej (rows 0:8 of EIJT)
        P[:, 8 + g] = Wg @ att_a[g, :HID]  # a_i -> ei
    import ml_dtypes
    Wb = W.astype(ml_dtypes.bfloat16)
    in_maps = []
    for core in range(CORES):
        hs = h[core * BPC:(core + 1) * BPC]            # [4, 512, 128]
        hTs = np.ascontiguousarray(hs.transpose(0, 2, 1))  # [4, 128, 512]
        in_maps.append({"hT": hTs, "hTb": hTs.astype(ml_dtypes.bfloat16),
                        "W": Wb, "P": P})
    return in_maps


def kernel(h, W, att_a):
    nc = _get_graph()
    in_maps = _prep_inputs(h, W, att_a)
    res = run_bass_kernel_spmd(nc, in_maps, list(range(CORES))).results
    outs = [r["out"].transpose(0, 2, 1, 3) for r in res]  # [4,H,n,d] each
    return np.ascontiguousarray(np.concatenate(outs, axis=0))
